# revision 15
# baseline (speedup 1.0000x reference)
"""CondConv (MoE-routing) block on 8 Trainium2 NeuronCores.

Computation per sample (see reference):
  x1 = relu(bn1(conv1x1(x, mix(r1(x), w1))))          256 -> 128 ch
  x2 = relu(bn2(dwconv3x3(x1, mix(r2(x1), w2))))      128 ch depthwise
  out = concat([x1, x2], ch)

Sharding: data-parallel over batch (32 samples -> 4 per core).

Per-core program (per sample):
  - conv1 as PE matmuls (fp32, exact): out1[o,hw] = sum_c k1T[c,o]*x[c,hw],
    K=256 in 2 partition tiles, N=3136 in 7 chunks of 448 (PSUM bank =
    512 fp32). BN1 scale folded into w1 host-side; ACT evacuates PSUM
    with fused bias+ReLU and accumulates pool2 sums per chunk.
  - routing: pooling on DVE (free-axis reduce), logits via tiny PE
    matmuls (weights pre-transposed, pre-scaled by 1/HW host-side),
    sigmoid on ACT, expert-mix on DVE.
  - depthwise 3x3 on PE in float32r (TF32-class, ~2e-4 rel err, ample
    for the absmax gate): per chunk, 9 accumulating diag-matmuls
    psum += diag(k2[:,t]) @ x1_shifted_t. x1 lives in a flat
    [128, 57+3136+57] buffer with zeroed guards so every shifted view
    is contiguous; 6 tiny DVE ops per chunk subtract the horizontal
    wrap at w=0 / w=55 from PSUM; ACT evacuates with fused BN2+ReLU.
"""

import os
import numpy as np

B, CIN, H, W = 32, 256, 56, 56
COUT = 256
INIT_C = 128
EXP_C = 128
NE = 4
BN_EPS = 1e-5
NCORES = 8
SPB = B // NCORES  # samples per core
HW = H * W  # 3136
GUARD = 57
NCHUNK = 7
CHUNK = HW // NCHUNK  # 448
ROWS = CHUNK // W  # 8 image rows per chunk

_DW_OFFS = [dh * W + dw for dh in (-1, 0, 1) for dw in (-1, 0, 1)]
_FIX_TAPS = [(0, 0), (3, 0), (6, 0), (2, W - 1), (5, W - 1), (8, W - 1)]

_prog_cache = {}


def _build_program():
    import concourse.bass as bass
    import concourse.tile as tile
    from concourse import mybir

    f32 = mybir.dt.float32
    f32r = mybir.dt.float32r
    AF = mybir.ActivationFunctionType
    ALU = mybir.AluOpType
    AX = mybir.AxisListType.X

    nc = bass.Bass("TRN2", target_bir_lowering=False, debug=False)

    x_d = nc.dram_tensor("x", [SPB, CIN, HW], f32r, kind="ExternalInput").ap()
    w1t_d = nc.dram_tensor("w1t", [2, NE, 128, 128], f32, kind="ExternalInput").ap()
    r1wt_d = nc.dram_tensor("r1wt", [CIN, NE], f32, kind="ExternalInput").ap()
    r1b_d = nc.dram_tensor("r1b", [1, NE], f32, kind="ExternalInput").ap()
    w2f_d = nc.dram_tensor("w2f", [NE, EXP_C, 9], f32, kind="ExternalInput").ap()
    r2wt_d = nc.dram_tensor("r2wt", [INIT_C, NE], f32, kind="ExternalInput").ap()
    r2b_d = nc.dram_tensor("r2b", [1, NE], f32, kind="ExternalInput").ap()
    bnb1_d = nc.dram_tensor("bnb1", [INIT_C, 1], f32, kind="ExternalInput").ap()
    bnb2_d = nc.dram_tensor("bnb2", [EXP_C, 1], f32, kind="ExternalInput").ap()
    ones_d = nc.dram_tensor("ones", [1, 128], f32, kind="ExternalInput").ap()
    ident_d = nc.dram_tensor("ident", [128, 128], f32, kind="ExternalInput").ap()
    out_d = nc.dram_tensor("out", [SPB, COUT, HW], f32, kind="ExternalOutput").ap()

    with tile.TileContext(nc) as tc:
        with (
            tc.tile_pool(name="weights", bufs=1) as wpool,
            tc.tile_pool(name="big", bufs=3) as bpool,
            tc.tile_pool(name="big2", bufs=2) as bpool2,
            tc.tile_pool(name="small", bufs=2) as spool,
            tc.tile_pool(name="pchunk", bufs=7, space="PSUM") as ppool,
            tc.tile_pool(name="psums", bufs=1, space="PSUM") as pspool,
        ):
            # ---- persistent weights ----
            w1t_sb = wpool.tile([128, 2 * NE * 128], f32, tag="w1t")
            nc.gpsimd.dma_start(
                w1t_sb[:].rearrange("p (g n) -> p g n", g=2 * NE),
                w1t_d[:].rearrange("j e p n -> p (j e) n"),
            )
            r1wt_a = wpool.tile([128, NE], f32, tag="r1wt_a")
            r1wt_b = wpool.tile([128, NE], f32, tag="r1wt_b")
            nc.gpsimd.dma_start(r1wt_a[:], r1wt_d[0:128, :])
            nc.gpsimd.dma_start(r1wt_b[:], r1wt_d[128:256, :])
            r1b_sb = wpool.tile([1, NE], f32, tag="r1b")
            nc.gpsimd.dma_start(r1b_sb[:], r1b_d[:])
            w2f_sb = wpool.tile([128, NE * 9], f32, tag="w2f")
            nc.gpsimd.dma_start(
                w2f_sb[:].rearrange("p (e n) -> p e n", e=NE),
                w2f_d[:].rearrange("e p n -> p e n"),
            )
            r2wt_sb = wpool.tile([128, NE], f32, tag="r2wt")
            nc.gpsimd.dma_start(r2wt_sb[:], r2wt_d[:])
            r2b_sb = wpool.tile([1, NE], f32, tag="r2b")
            nc.gpsimd.dma_start(r2b_sb[:], r2b_d[:])
            bnb1_sb = wpool.tile([128, 1], f32, tag="bnb1")
            nc.gpsimd.dma_start(bnb1_sb[:], bnb1_d[:])
            bnb2_sb = wpool.tile([128, 1], f32, tag="bnb2")
            nc.gpsimd.dma_start(bnb2_sb[:], bnb2_d[:])
            ones_sb = wpool.tile([1, 128], f32, tag="ones")
            nc.gpsimd.dma_start(ones_sb[:], ones_d[:])
            ident_sb = wpool.tile([128, 128], f32, tag="ident")
            nc.gpsimd.dma_start(ident_sb[:], ident_d[:])

            for s in range(SPB):
                # ---- load x shard ----
                xa = bpool.tile([128, HW], f32r, tag="xa")
                nc.sync.dma_start(xa[:], x_d[s, 0:128, :])
                xb = bpool.tile([128, HW], f32r, tag="xb")
                nc.sync.dma_start(xb[:], x_d[s, 128:256, :])

                # ---- pool1 (sums; 1/HW folded into r1wt) ----
                # xa on DVE; xb via ACT Copy+accum_out into x1flat (dead until BN1)
                x1flat = bpool.tile([128, HW], f32r, tag="x1flat")
                p1 = spool.tile([128, 2], f32, tag="p1")
                nc.vector.reduce_sum(p1[:, 0:1], xa[:], AX)
                nc.scalar.activation(x1flat[:], xb[:], AF.Copy, accum_out=p1[:, 1:2])

                # ---- routing 1 ----
                ps_r = pspool.tile([128, NE], f32, tag="ps_small", name="ps_r")
                nc.tensor.matmul(ps_r[0:1, :], p1[:, 0:1], r1wt_a[:], start=True, stop=False)
                nc.tensor.matmul(ps_r[0:1, :], p1[:, 1:2], r1wt_b[:], start=False, stop=True)
                r1s = spool.tile([1, NE], f32, tag="r1s")
                nc.vector.tensor_tensor(r1s[:], ps_r[0:1, :], r1b_sb[:], op=ALU.add)
                nc.scalar.activation(r1s[:], r1s[:], AF.Sigmoid)
                ps_rb = pspool.tile([128, NE], f32, tag="ps_small", name="ps_rb")
                nc.tensor.matmul(ps_rb[:], ones_sb[:], r1s[:], start=True, stop=True)
                rb = spool.tile([128, NE], f32, tag="rb")
                nc.vector.tensor_copy(rb[:], ps_rb[:])

                # ---- mix k1T = sum_e rb[:,e] * w1t[j,e]  (DVE) ----
                k1t = spool.tile([128, 256], f32r, tag="k1t")
                for j in range(2):
                    dst = k1t[:, j * 128 : (j + 1) * 128]
                    w_of = lambda e: w1t_sb[:, (j * NE + e) * 128 : (j * NE + e + 1) * 128]
                    nc.vector.tensor_scalar(dst, w_of(0), rb[:, 0:1], None, ALU.mult)
                    for e in range(1, NE):
                        nc.vector.scalar_tensor_tensor(
                            dst, w_of(e), rb[:, e : e + 1], dst, ALU.mult, ALU.add
                        )

                # ---- conv1 + BN1 + ReLU into flat x1 ----
                p2cols = spool.tile([128, NCHUNK], f32, tag="p2cols")
                pchunks = [
                    ppool.tile([128, CHUNK], f32, tag="pchunk", name=f"c1_{s}_{n}")
                    for n in range(NCHUNK)
                ]
                for j, xt in ((0, xa), (1, xb)):
                    for n in range(NCHUNK):
                        nc.tensor.matmul(
                            pchunks[n][:],
                            k1t[:, j * 128 : (j + 1) * 128],
                            xt[:, n * CHUNK : (n + 1) * CHUNK],
                            start=(j == 0), stop=(j == 1),
                        )
                for n in range(NCHUNK):
                    nc.scalar.activation(
                        x1flat[:, n * CHUNK : (n + 1) * CHUNK],
                        pchunks[n][:], AF.Relu, bias=bnb1_sb[:],
                        accum_out=p2cols[:, n : n + 1],
                    )
                nc.gpsimd.dma_start(out_d[s, 0:INIT_C, :], x1flat[:])
                # 58-wide zero-padded copy for the PE depthwise reads (GPSIMD)
                xpad = bpool2.tile([128, 58 * 58], f32r, tag="xpad")
                xpad_r = xpad[:].rearrange("p (r c) -> p r c", c=58)
                nc.gpsimd.memset(xpad[:, 0:58].bitcast(f32), 0.0)
                nc.gpsimd.memset(xpad[:, 57 * 58 :].bitcast(f32), 0.0)
                nc.gpsimd.memset(xpad_r[:, 1:57, 0:1].bitcast(f32), 0.0)
                nc.gpsimd.memset(xpad_r[:, 1:57, 57:58].bitcast(f32), 0.0)
                nc.gpsimd.tensor_copy(
                    xpad_r[:, 1:57, 1:57],
                    x1flat[:].rearrange("p (h w) -> p h w", w=W),
                )

                # ---- routing 2 ----
                p2 = spool.tile([128, 1], f32, tag="p2")
                nc.vector.reduce_sum(p2[:], p2cols[:], AX)
                ps_r2 = pspool.tile([128, NE], f32, tag="ps_small", name="ps_r2")
                nc.tensor.matmul(ps_r2[0:1, :], p2[:], r2wt_sb[:], start=True, stop=True)
                r2s = spool.tile([1, NE], f32, tag="r2s")
                nc.vector.tensor_tensor(r2s[:], ps_r2[0:1, :], r2b_sb[:], op=ALU.add)
                nc.scalar.activation(r2s[:], r2s[:], AF.Sigmoid)
                ps_rb2 = pspool.tile([128, NE], f32, tag="ps_small", name="ps_rb2")
                nc.tensor.matmul(ps_rb2[:], ones_sb[:], r2s[:], start=True, stop=True)
                rb2 = spool.tile([128, NE], f32, tag="rb2")
                nc.vector.tensor_copy(rb2[:], ps_rb2[:])

                # ---- mix k2 [128, 9]; negated copy for the wrap fix ----
                k2 = spool.tile([128, 9], f32, tag="k2")
                nc.vector.tensor_scalar(k2[:], w2f_sb[:, 0:9], rb2[:, 0:1], None, ALU.mult)
                for e in range(1, NE):
                    nc.vector.scalar_tensor_tensor(
                        k2[:], w2f_sb[:, e * 9 : (e + 1) * 9], rb2[:, e : e + 1], k2[:],
                        ALU.mult, ALU.add,
                    )
                # ---- diag expert kernels for the PE dwconv ----
                diag = spool.tile([128, 9 * 128], f32r, tag="diag")
                for t in range(9):
                    nc.vector.tensor_scalar(
                        diag[:, t * 128 : (t + 1) * 128], ident_sb[:],
                        k2[:, t : t + 1], None, ALU.mult,
                    )

                # ---- depthwise conv on PE (float32r) + BN2 + ReLU ----
                x2 = bpool2.tile([128, HW], f32, tag="x2")
                for n in range(NCHUNK):
                    ps2 = ppool.tile([128, CHUNK], f32, tag="pchunk", name=f"dw_{s}_{n}")
                    for t in range(9):
                        dh, dw = t // 3 - 1, t % 3 - 1
                        rhs = xpad_r[:, n * ROWS + dh + 1 : n * ROWS + dh + 9, dw + 1 : dw + 57]
                        nc.tensor.matmul(
                            ps2[:], diag[:, t * 128 : (t + 1) * 128], rhs,
                            start=(t == 0), stop=(t == 8),
                        )
                    nc.scalar.activation(
                        x2[:, n * CHUNK : (n + 1) * CHUNK], ps2[:], AF.Relu,
                        bias=bnb2_sb[:],
                    )
                nc.gpsimd.dma_start(out_d[s, INIT_C:COUT, :], x2[:])

    return nc


def _host_prep(x, r1_w, r1_b, w1, g1, b1, m1, v1, r2_w, r2_b, w2, g2, b2, m2, v2):
    inv1 = g1 / np.sqrt(v1 + BN_EPS)
    inv2 = g2 / np.sqrt(v2 + BN_EPS)
    bnb1 = (b1 - m1 * inv1).reshape(INIT_C, 1).astype(np.float32)
    bnb2 = (b2 - m2 * inv2).reshape(EXP_C, 1).astype(np.float32)
    # w1: [E, O, C, 1, 1] -> fold inv1 over O -> w1t[j, e, c_local, o]
    w1s = w1[:, :, :, 0, 0] * inv1[None, :, None]  # [E, O, C]
    w1t = np.ascontiguousarray(
        w1s.transpose(2, 0, 1).reshape(2, 128, NE, 128).transpose(0, 2, 1, 3)
    ).astype(np.float32)  # [2, E, 128c, 128o]
    # w2: [E, C, 1, 3, 3] -> fold inv2 over C -> [E, C, 9]
    w2f = (w2[:, :, 0, :, :] * inv2[None, :, None, None]).reshape(NE, EXP_C, 9)
    w2f = np.ascontiguousarray(w2f).astype(np.float32)
    common = {
        "w1t": w1t,
        "r1wt": np.ascontiguousarray(r1_w.T / HW).astype(np.float32),
        "r1b": r1_b.reshape(1, NE).astype(np.float32),
        "w2f": w2f,
        "r2wt": np.ascontiguousarray(r2_w.T / HW).astype(np.float32),
        "r2b": r2_b.reshape(1, NE).astype(np.float32),
        "bnb1": bnb1,
        "bnb2": bnb2,
        "ones": np.ones((1, 128), dtype=np.float32),
        "ident": np.eye(128, dtype=np.float32),
    }
    return common


def kernel(**inputs):
    x = np.asarray(inputs["x"], dtype=np.float32)
    common = _host_prep(**{k: np.asarray(v) for k, v in inputs.items()})

    if "nc" not in _prog_cache:
        _prog_cache["nc"] = _build_program()
    nc = _prog_cache["nc"]
    sim_mode = bool(os.environ.get("BASS_KERNEL_SIM"))
    if not sim_mode and not _prog_cache.get("fixed"):
        from waitfix import fix_sync
        fix_sync(nc)
        _prog_cache["fixed"] = True

    xs = x.reshape(NCORES, SPB, CIN, HW)
    in_maps = [dict(common, x=np.ascontiguousarray(xs[c])) for c in range(NCORES)]

    if sim_mode:
        from concourse.bass_interp import CoreSim

        sim = CoreSim(nc)
        for name, arr in in_maps[0].items():
            sim.tensor(name)[:] = arr
        sim.simulate()
        out = np.zeros((NCORES, SPB, COUT, HW), dtype=np.float32)
        out[0] = sim.tensor("out")
        return out.reshape(B, COUT, H, W)

    from concourse.bass_utils import run_bass_kernel_spmd

    res = run_bass_kernel_spmd(nc, in_maps, list(range(NCORES)))
    _prog_cache["last_results"] = res
    out = np.stack([res.results[c]["out"] for c in range(NCORES)])
    return out.reshape(B, COUT, H, W)
